# revision 59
# baseline (speedup 1.0000x reference)
"""CSWin Transformer block kernel for 8 Trainium2 NeuronCores.

Data-parallel over batch: 32 images -> 4 per core. Each core runs the full
block (LN1, qkv, cross-shaped window attention with LePE, proj, residual,
LN2, MLP, residual) on its shard, fully pipelined per image.

Layouts per core (T = 4*3136 = 12544 tokens):
  - token-major: (112 tokens on partitions, 128 ch free), 28 tiles per image.
  - channel-major: (128 ch on partitions, tokens free).
  - Branch 0 (56x2 column stripes) tokens are kept in w-major order
    (p = 56*w + h) in rows 0:64 of channel-major tensors; branch 1 rows
    64:128 use h-major (t = 56*h + w). Window w of either branch is then
    columns [112*w, 112*w+112).

LN gammas folded into the following matmul weights host-side; LN betas enter
as constant rows via per-partition bias adds on C-major evacuations.
LePE conv bias + the v-bias row are folded into the proj bias.
"""
import sys
sys.path.insert(0, "/opt/trn_rl_repo")
import os
import numpy as np
import concourse.bass as bass
from concourse import bacc
import concourse.tile as tile
from concourse import mybir
from concourse.bass_utils import run_bass_kernel_spmd
from concourse.masks import make_identity

F32 = mybir.dt.float32
BF16 = mybir.dt.bfloat16
AL = mybir.AluOpType
AF = mybir.ActivationFunctionType

N_CORES = 8
B, RESO, C = 32, 56, 128
L = RESO * RESO            # 3136
IMG = B // N_CORES         # 4 images per core
T = IMG * L                # 12544 tokens per core
PT = 112                   # tokens per token-major tile
NTI = L // PT              # 28 token tiles per image
CK = 448                   # tokens per C-major chunk
NCK = L // CK              # 7 chunks per image
NWIN = 28                  # windows per image per branch
WT = 112                   # tokens per window
HD = 32
EPS = 1e-5


def build(nc, dbg=()):
    x_in = nc.declare_dram_parameter("x", [T, C], F32, isOutput=False)
    wqkv_in = nc.declare_dram_parameter("wqkv", [C, 3 * C], F32, isOutput=False)
    wproj_in = nc.declare_dram_parameter("wproj", [C, C], F32, isOutput=False)
    wfc1_in = nc.declare_dram_parameter("wfc1", [C, 4 * C], F32, isOutput=False)
    wfc2_in = nc.declare_dram_parameter("wfc2", [4 * C, C], F32, isOutput=False)
    # vecs cols: 0:s2q 1:s2k 2:s2v 3:projb 4:fc2b 5:eps 6..14:taps 15..18:fc1b
    vecs_in = nc.declare_dram_parameter("vecs", [C, 19], F32, isOutput=False)
    out_t = nc.declare_dram_parameter("out", [T, C], F32, isOutput=True)
    dbg_outs = {}
    for name, shape in dbg:
        dbg_outs[name] = nc.declare_dram_parameter(name, shape, F32, isOutput=True)

    tc = tile.TileContext(nc)
    with tc:
        with (
            tc.tile_pool(name="consts", bufs=1) as consts,
            tc.tile_pool(name="glob", bufs=1) as glob,
            tc.tile_pool(name="pimg", bufs=2) as pimg,
            tc.tile_pool(name="small", bufs=2) as small,
            tc.tile_pool(name="psU", bufs=3, space="PSUM") as psU,
            tc.tile_pool(name="psT", bufs=2, space="PSUM") as psT,
        ):
            _body(nc, consts, glob, pimg, small, psU, psT,
                  x_in, wqkv_in, wproj_in, wfc1_in, wfc2_in, vecs_in,
                  out_t, dbg_outs)
    return nc


def _body(nc, consts, glob, pimg, small, psU, psT,
          x_in, wqkv_in, wproj_in, wfc1_in, wfc2_in, vecs_in, out_t, dbg_outs):
    # ---------------- constants / weights ----------------
    identb = consts.tile([128, 128], BF16)
    make_identity(nc, identb[:])
    onesb = consts.tile([WT, 32], BF16)
    nc.vector.memset(onesb[:], 1.0)
    wqkv = consts.tile([C, 3 * C], BF16)
    nc.gpsimd.dma_start(out=wqkv[:], in_=wqkv_in[:])
    wproj = consts.tile([C, C], BF16)
    nc.gpsimd.dma_start(out=wproj[:], in_=wproj_in[:])
    wfc1 = consts.tile([C, 4 * C], BF16)
    nc.gpsimd.dma_start(out=wfc1[:], in_=wfc1_in[:])
    wfc2 = consts.tile([C, 4, C], BF16)
    nc.gpsimd.dma_start(out=wfc2[:], in_=wfc2_in.rearrange("(k p) o -> p k o", p=C))
    vecs = consts.tile([C, 19], F32)
    nc.sync.dma_start(out=vecs[:], in_=vecs_in[:])
    s2q, s2k, s2v = vecs[:, 0:1], vecs[:, 1:2], vecs[:, 2:3]
    projb, fc2b, epsv = vecs[:, 3:4], vecs[:, 4:5], vecs[:, 5:6]
    taps = [vecs[:, 6 + i:7 + i] for i in range(9)]
    fc1b = [vecs[:, 15 + h:16 + h] for h in range(4)]

    # block-diagonal q: bd_q[p, win, h, :] = q[p, win cols] iff p//32 == h,
    # so ONE matmul per window computes all 4 heads' QK^T (the zeros kill
    # cross-head/cross-branch contraction terms). Diagonal blocks are
    # refilled per image by SBUF->SBUF DMA from the dual-order qT (whose
    # window columns are contiguous); the off-diagonal zeros are written
    # once here and never touched again.
    bd_q = glob.tile([C, NWIN, 4, WT], BF16)
    nc.gpsimd.memset(bd_q[:], 0.0)

    def prefetch_x(img):
        # x in token-major tiles (tile ti = tokens [112*ti, +112) of this image)
        x_tm = pimg.tile([PT, NTI, C], F32, tag="x_img", bufs=3)
        nc.sync.dma_start(
            out=x_tm[:, :, :],
            in_=x_in[img * L:(img + 1) * L].rearrange("(n p) c -> p n c", p=PT))
        return x_tm

    def proj_res1_chunk(ck, stPR):
        # proj + residual-1 for one 448-token chunk of the PREVIOUS image;
        # interwoven with the next image's LN1 groups so the tensor engine
        # has ready matmuls while LN1's vector work gates its transposes.
        x_tm, att_h, sum1, sumsq = stPR
        sl = bass.ts(ck, CK)
        pp = psU.tile([C, 2, 512], F32, tag="u")
        nc.tensor.matmul(pp[:, 0, 0:CK], wproj[:], att_h[:, sl],
                         start=True, stop=True)
        projTc = small.tile([C, CK], BF16, tag="projTc", bufs=3)
        nc.scalar.activation(projTc[:], pp[:, 0, 0:CK], AF.Identity, bias=projb)
        for tj in range(4):
            ti = 4 * ck + tj
            ptp = psT.tile([PT, C], BF16, tag="tp")
            nc.tensor.transpose(ptp[:], projTc[:, bass.ts(tj, PT)], identb[:, 0:C])
            nc.vector.scalar_tensor_tensor(out=x_tm[:, ti, :], in0=ptp[:],
                                           scalar=1.0, in1=x_tm[:, ti, :],
                                           op0=AL.mult, op1=AL.add,
                                           accum_out=sum1[:, ti:ti + 1])
            sq = small.tile([PT, C], F32, tag="sqt")
            nc.scalar.activation(sq[:], x_tm[:, ti, :], AF.Square,
                                 accum_out=sumsq[:, ti:ti + 1])

    def alloc_mvs():
        return small.tile([PT, NTI, 2], F32, tag="mvs", name="mvs")

    def ln1_stats_group(tq, x_tm, mvs):
        # LN1 bn stats for 4 token tiles; hosted inside the previous
        # image's MLP chunk loop, where the vector engine otherwise idles
        for ti in range(4 * tq, 4 * tq + 4):
            st = small.tile([PT, 6], F32, tag="bnst")
            nc.vector.bn_stats(out=st[:], in_=x_tm[:, ti, :])
            nc.vector.bn_aggr(out=mvs[:, ti, :], in_=st[:])

    def phase_A1(img, x_tm, stPR=None, mvs=None):
        base_t = 0

        # ---- LN1 stats + apply + transpose ----
        if mvs is None:
            # image 0: no preceding B2 hosted the stats; compute inline
            mvs = alloc_mvs()
            for tq in range(NCK):
                ln1_stats_group(tq, x_tm, mvs)
        rstd = small.tile([PT, NTI], F32, tag="rstd")
        lnx_h = pimg.tile([C, L], BF16, tag="lnx_h")
        lnx_w = pimg.tile([C, L], BF16, tag="lnx_w")
        lnx_h_v = lnx_h.rearrange("p (n t) -> p n t", t=PT)
        for tg in range(NTI // 7):
            gsl = bass.ds(7 * tg, 7)
            nc.scalar.activation(rstd[:, gsl], mvs[:, gsl, 1], AF.Ln,
                                 bias=epsv[0:PT, :])
            nc.scalar.activation(rstd[:, gsl], rstd[:, gsl], AF.Exp, scale=-0.5)
        # w-major reorder views: p = 56*w + h  <-  t = 56*h + w. The copy is
        # chunked along h (the freshly-evacuated columns), so each reorder
        # piece runs right after its 4-tile group and the whole reorder
        # finishes with LN1 instead of serializing after it.
        lnw_v = lnx_w.rearrange("p (w h) -> p w h", w=RESO)
        lnh_v = lnx_h.rearrange("p (h w) -> p h w", h=RESO).rearrange("p h w -> p w h")
        for tq in range(NTI // 4):
            ztp = psT.tile([C, 4, PT], BF16, tag="tp")
            for j in range(4):
                ti = 4 * tq + j
                z = small.tile([PT, C], BF16, tag="zt")
                nc.vector.tensor_scalar(out=z[:], in0=x_tm[:, base_t + ti, :],
                                        scalar1=mvs[:, ti, 0:1],
                                        scalar2=rstd[:, ti:ti + 1],
                                        op0=AL.subtract, op1=AL.mult)
                nc.tensor.transpose(ztp[:, j, :], z[:], identb[0:PT, 0:PT])
            nc.vector.tensor_copy(lnx_h_v[:, bass.ds(4 * tq, 4), :], ztp[:, :, :])
            hv = bass.ds(8 * tq, 8)
            nc.gpsimd.tensor_copy(out=lnw_v[:, :, hv], in_=lnh_v[:, :, hv])
            if stPR is not None:
                proj_res1_chunk(tq, stPR)

        # ---- qkv (col-packed: br0 from lnx_w -> rows 0:64, br1 from lnx_h).
        # q only feeds the bd_q diagonal refill, so it lives in a small
        # per-chunk tile and the DMAs (idle DMA engines, ~25ns Pool-sequencer
        # dispatch) are issued inline ----
        kT = pimg.tile([C, L], BF16, tag="kT")
        vT = pimg.tile([C, L], BF16, tag="vT")
        for ck in range(NCK):
            sl = bass.ts(ck, CK)
            pqk = psU.tile([C, 2, 512], F32, tag="u")
            pv = psU.tile([C, 2, 512], F32, tag="u")
            for half, src in ((0, lnx_w), (1, lnx_h)):
                hs = bass.ds(64 * half, 64)
                nc.tensor.matmul(pqk[hs, 0, 0:CK], wqkv[:, bass.ds(64 * half, 64)],
                                 src[:, sl], start=True, stop=True,
                                 tile_position=(0, 64 * half))
                nc.tensor.matmul(pqk[hs, 1, 0:CK], wqkv[:, bass.ds(C + 64 * half, 64)],
                                 src[:, sl], start=True, stop=True,
                                 tile_position=(0, 64 * half))
                nc.tensor.matmul(pv[hs, 0, 0:CK], wqkv[:, bass.ds(2 * C + 64 * half, 64)],
                                 src[:, sl], start=True, stop=True,
                                 tile_position=(0, 64 * half))
            qTc = small.tile([C, CK], BF16, tag="qTc", bufs=3)
            nc.scalar.activation(qTc[:], pqk[:, 0, 0:CK], AF.Identity, bias=s2q)
            nc.scalar.activation(kT[:, sl], pqk[:, 1, 0:CK], AF.Identity, bias=s2k)
            nc.vector.tensor_scalar(out=vT[:, sl], in0=pv[:, 0, 0:CK], scalar1=s2v,
                                    scalar2=None, op0=AL.add)
            for h in range(4):
                hp = bass.ds(32 * h, 32)
                nc.gpsimd.dma_start(
                    out=bd_q[hp, bass.ds(4 * ck, 4), h, :],
                    in_=qTc[hp, :].rearrange("p (g q) -> p g q", g=4))

        # ---- v_tm: token-major v via transposes of dual-order vT (window
        # columns are contiguous), 4 windows per psum bank ----
        v_tm = pimg.tile([PT, NWIN, C], BF16, tag="v_tm")
        for r in range(NWIN // 4):
            pvt = psT.tile([PT, 4, C], BF16, tag="tp")
            for g in range(4):
                nc.tensor.transpose(pvt[:, g, :], vT[:, bass.ts(4 * r + g, WT)],
                                    identb[:])
            nc.vector.tensor_copy(v_tm[:, bass.ds(4 * r, 4), :], pvt[:, :, :])
        return x_tm, kT, vT, v_tm

    def phase_A2(img, stateA1):
        x_tm, kT, vT, v_tm = stateA1
        # ---- attention: per superchunk (4 windows); one QK matmul per
        # window computes all 4 heads via the block-diagonal q. LePE taps
        # and the h-major reorder are chunked per superchunk too (windows
        # are independent for the depthwise conv), so they pipeline under
        # later superchunks' matmuls instead of serializing at the end ----
        tmp_att = pimg.tile([C, L], BF16, tag="tmp_att")
        att_h = pimg.tile([C, L], BF16, tag="att_h")
        va = vT.rearrange("p (s y x) -> p s y x", s=NWIN, y=2)
        aa = tmp_att.rearrange("p (s y x) -> p s y x", s=NWIN, y=2)
        ah_v = att_h[0:64].rearrange("p (h w) -> p h w", h=RESO)
        ta_v = tmp_att[0:64].rearrange("p (w h) -> p w h", w=RESO).rearrange(
            "p w h -> p h w")

        def tap(dy, dx, ssl):
            t = taps[(dy + 1) * 3 + (dx + 1)]
            xo0, xo1 = max(0, -dx), max(0, dx)
            xn = RESO - abs(dx)
            if dy == 0:
                o = aa[:, ssl, :, bass.ds(xo0, xn)]
                i = va[:, ssl, :, bass.ds(xo1, xn)]
            elif dy == 1:
                o = aa[:, ssl, 0:1, bass.ds(xo0, xn)]
                i = va[:, ssl, 1:2, bass.ds(xo1, xn)]
            else:
                o = aa[:, ssl, 1:2, bass.ds(xo0, xn)]
                i = va[:, ssl, 0:1, bass.ds(xo1, xn)]
            nc.vector.scalar_tensor_tensor(out=o, in0=i, scalar=t, in1=o,
                                           op0=AL.mult, op1=AL.add)

        for s4 in range(NCK):
            # pT_t[tk, h, win-in-s4, tq]
            pT_t = small.tile([WT, 4, 4, WT], BF16, tag="pT")
            for wp in range(2):            # window pairs
                sp = psU.tile([WT, 2, 512], F32, tag="u")
                for g2 in range(2):
                    win = 4 * s4 + 2 * wp + g2
                    nc.tensor.matmul(sp[:, g2, 0:4 * WT], kT[:, bass.ts(win, WT)],
                                     bd_q[:, win, :, :], start=True, stop=True)
                # psum cols are (h, tq) per window; view to pT_t (h, win, tq)
                spv = sp[:, :, 0:4 * WT].rearrange("p w (h q) -> p h w q", h=4)
                nc.scalar.activation(pT_t[:, :, bass.ds(2 * wp, 2), :],
                                     spv, AF.Exp)
            sa = psU.tile([C, 2, 512], F32, tag="u")
            sums = sa[:, 0, 0:CK]
            avp = sa[:, 1, 0:CK]
            for h in range(4):
                po = bass.ds(32 * h, 32)
                nc.tensor.matmul(sa[po, 0, 0:CK], onesb[:, 0:32], pT_t[:, h, :, :],
                                 start=True, stop=True, tile_position=(0, 32 * h))
                vsl = bass.ds(64 * (h // 2) + 32 * (h % 2), 32)
                for g in range(4):
                    nc.tensor.matmul(sa[po, 1, bass.ds(112 * g, WT)],
                                     v_tm[:, 4 * s4 + g, vsl],
                                     pT_t[:, h, g, :],
                                     start=True, stop=True,
                                     tile_position=(0, 32 * h))
            lg = small.tile([C, CK], F32, tag="lg")
            nc.scalar.activation(lg[:], sums, AF.Ln)
            rec = small.tile([C, CK], F32, tag="rec")
            nc.scalar.activation(rec[:], lg[:], AF.Exp, scale=-1.0)
            nc.vector.tensor_tensor(out=tmp_att[:, bass.ts(s4, CK)], in0=avp,
                                    in1=rec[:], op=AL.mult)
            # LePE taps for these 4 stripes, then reorder them to h-major
            ssl = bass.ds(4 * s4, 4)
            for dy in (0, 1, -1):
                for dx in (0, 1, -1):
                    tap(dy, dx, ssl)
            wv = bass.ds(8 * s4, 8)
            nc.gpsimd.tensor_copy(out=ah_v[:, :, wv], in_=ta_v[:, :, wv])
            nc.gpsimd.tensor_copy(out=att_h[64:128, bass.ts(s4, CK)],
                                  in_=tmp_att[64:128, bass.ts(s4, CK)])

        if "attT" in dbg_outs and img == 0:
            dc = small.tile([C, L], F32, tag="dbg")
            nc.vector.tensor_copy(dc[:], att_h[:])
            nc.sync.dma_start(out=dbg_outs["attT"], in_=dc[:])
        return x_tm, att_h

    def phase_B1(img, state, stPR):
        x_tm, att_h = state
        _, _, sum1, sumsq = stPR
        base_t = 0
        # rstd2 = 1/sqrt(sumsq/C - (sum1/C)^2 + eps); mean2 = sum1/C
        mean2 = small.tile([PT, NTI], F32, tag="mean2")
        var2 = small.tile([PT, NTI], F32, tag="var2")
        rstd2 = small.tile([PT, NTI], F32, tag="rstd2")
        lnx2 = pimg.tile([C, L], BF16, tag="lnx2")
        for tg in range(NTI // 7):
            gsl = bass.ds(7 * tg, 7)
            nc.vector.tensor_scalar(out=mean2[:, gsl], in0=sum1[:, gsl],
                                    scalar1=1.0 / C, scalar2=None, op0=AL.mult)
            nc.vector.tensor_tensor(out=var2[:, gsl], in0=mean2[:, gsl],
                                    in1=mean2[:, gsl], op=AL.mult)
            nc.vector.scalar_tensor_tensor(out=var2[:, gsl], in0=sumsq[:, gsl],
                                           scalar=1.0 / C, in1=var2[:, gsl],
                                           op0=AL.mult, op1=AL.subtract)
            nc.scalar.activation(rstd2[:, gsl], var2[:, gsl], AF.Ln,
                                 bias=epsv[0:PT, :])
            nc.scalar.activation(rstd2[:, gsl], rstd2[:, gsl], AF.Exp, scale=-0.5)
        lnx2_v = lnx2.rearrange("p (n t) -> p n t", t=PT)
        for tq in range(NTI // 4):
            z2p = psT.tile([C, 4, PT], BF16, tag="tp")
            for j in range(4):
                ti = 4 * tq + j
                z2 = small.tile([PT, C], BF16, tag="z2t")
                nc.vector.tensor_scalar(out=z2[:], in0=x_tm[:, base_t + ti, :],
                                        scalar1=mean2[:, ti:ti + 1],
                                        scalar2=rstd2[:, ti:ti + 1],
                                        op0=AL.subtract, op1=AL.mult)
                nc.tensor.transpose(z2p[:, j, :], z2[:], identb[0:PT, 0:PT])
            nc.vector.tensor_copy(lnx2_v[:, bass.ds(4 * tq, 4), :], z2p[:, :, :])
        return lnx2

    def phase_B2(img, x_tm, lnx2, nxt=None):
        base_t = 0
        # ---- MLP + residual 2 (hosting the NEXT image's LN1 stats) ----
        for ck in range(NCK):
            if nxt is not None:
                ln1_stats_group(ck, nxt[0], nxt[1])
            sl = bass.ts(ck, CK)
            hb = small.tile([C, 4, CK], BF16, tag="hb")
            for hp in range(2):
                ph = psU.tile([C, 2, 512], F32, tag="u")
                for hh in range(2):
                    h = 2 * hp + hh
                    nc.tensor.matmul(ph[:, hh, 0:CK], wfc1[:, bass.ds(128 * h, 128)],
                                     lnx2[:, sl], start=True, stop=True)
                    nc.scalar.activation(hb[:, h, :], ph[:, hh, 0:CK], AF.Gelu,
                                         bias=fc1b[h])
            p2 = psU.tile([C, 2, 512], F32, tag="u")
            for h in range(4):
                nc.tensor.matmul(p2[:, 0, 0:CK], wfc2[:, h, :], hb[:, h, :],
                                 start=(h == 0), stop=(h == 3))
            f2 = small.tile([C, CK], BF16, tag="f2")
            nc.scalar.activation(f2[:], p2[:, 0, 0:CK], AF.Identity, bias=fc2b)
            ftp = psT.tile([PT, 4, C], BF16, tag="tp")
            for tj in range(4):
                nc.tensor.transpose(ftp[:, tj, :], f2[:, bass.ts(tj, PT)],
                                    identb[:, 0:C])
            xsl = x_tm[:, bass.ds(base_t + 4 * ck, 4), :]
            nc.vector.tensor_tensor(out=xsl, in0=ftp[:, :, :], in1=xsl, op=AL.add)

        # out on the Activation HWDGE queue: keeps the sync queue free for
        # the x prefetches so neither stream head-of-line blocks the other
        nc.scalar.dma_start(
            out=out_t[img * L:(img + 1) * L].rearrange("(n p) c -> p n c", p=PT),
            in_=x_tm[:, :, :])

    # Skewed software pipeline, zippered at half-phase granularity: each
    # step emits A1(i), B1(i-1), A2(i), B2(i-1) so every engine's in-order
    # queue alternates between independent work from adjacent images —
    # halving head-of-line blocking versus whole-phase interleaving. The
    # scalar stream still ends each step with the gelu block, so the
    # exp/ln <-> gelu activation-table swap stays at 2 loads per image.
    stA1 = [None] * IMG
    stA2 = [None] * IMG
    stB1 = [None] * IMG
    stPR = [None] * IMG
    mvs_arr = [None] * IMG
    xs = [None] * IMG
    xs[0] = prefetch_x(0)
    for i in range(IMG + 1):
        if i >= 1:
            # proj/res1 state for image i-1 (filled chunk-wise inside A1(i))
            sum1 = small.tile([PT, NTI], F32, tag="sum1", name="sum1")
            sumsq = small.tile([PT, NTI], F32, tag="sumsq", name="sumsq")
            stPR[i - 1] = (stA2[i - 1][0], stA2[i - 1][1], sum1, sumsq)
        if i < IMG:
            stA1[i] = phase_A1(i, xs[i], stPR[i - 1] if i >= 1 else None,
                               mvs=mvs_arr[i])
            # prefetch the next image's x now; with 3 x buffers the slot
            # being refilled belongs to image i-2, whose B2 finished a full
            # step ago, so the DMA starts immediately
            if i + 1 < IMG:
                xs[i + 1] = prefetch_x(i + 1)
        else:
            # last image has no following A1 to host its proj/res1 chunks
            for ck in range(NCK):
                proj_res1_chunk(ck, stPR[i - 1])
        if i >= 1:
            stB1[i - 1] = phase_B1(i - 1, stA2[i - 1], stPR[i - 1])
        if i < IMG:
            stA2[i] = phase_A2(i, stA1[i])
        if i >= 1:
            nxt = None
            if i + 1 < IMG:
                # B2(i-1) hosts LN1 stats for image i+1 (whose x is already
                # prefetched), filling vector-engine idle under the MLP
                mvs_arr[i + 1] = alloc_mvs()
                nxt = (xs[i + 1], mvs_arr[i + 1])
            phase_B2(i - 1, stA2[i - 1][0], stB1[i - 1], nxt)


def _prep_inputs(inputs):
    """Host-side weight preprocessing (fp64 for exact folds)."""
    g1 = inputs["norm1_g"].astype(np.float64)
    b1 = inputs["norm1_b"].astype(np.float64)
    g2 = inputs["norm2_g"].astype(np.float64)
    b2 = inputs["norm2_b"].astype(np.float64)
    qkv_w = inputs["qkv_w"].astype(np.float64)
    proj_w = inputs["proj_w"].astype(np.float64)
    fc1_w = inputs["fc1_w"].astype(np.float64)
    fc2_w = inputs["fc2_w"].astype(np.float64)
    scale = HD ** -0.5

    wqkv = g1[:, None] * qkv_w
    s2 = b1 @ qkv_w
    wqkv[:, 0:C] *= scale
    s2q = s2[0:C] * scale
    s2k = s2[C:2 * C]
    s2v = s2[2 * C:3 * C]

    # LePE taps in stripe coords (y = stripe row in {0,1}, x = along stripe):
    # br1 (rows 64:128, h-major): (y,x) = (img_y, img_x) -> w1[dy+1, dx+1]
    # br0 (rows 0:64, w-major):  (y,x) = (img_x, img_y) -> transposed kernel
    w0 = inputs["conv_w0"].astype(np.float64)[:, 0]
    w1 = inputs["conv_w1"].astype(np.float64)[:, 0]
    taps = np.zeros((C, 9))
    for dy in (-1, 0, 1):
        for dx in (-1, 0, 1):
            ti = (dy + 1) * 3 + (dx + 1)
            taps[0:64, ti] = w0[:, dx + 1, dy + 1]
            taps[64:128, ti] = w1[:, dy + 1, dx + 1]

    # v_tm is transposed from vT, which already carries s2v, so unlike the
    # conv bias it must not be folded into the proj bias here.
    cb = np.concatenate([inputs["conv_b0"], inputs["conv_b1"]]).astype(np.float64)
    projb_eff = inputs["proj_b"].astype(np.float64) + cb @ proj_w

    wfc1 = g2[:, None] * fc1_w
    fc1b_eff = b2 @ fc1_w + inputs["fc1_b"].astype(np.float64)

    vecs = np.zeros((C, 19))
    vecs[:, 0], vecs[:, 1], vecs[:, 2] = s2q, s2k, s2v
    vecs[:, 3], vecs[:, 4] = projb_eff, inputs["fc2_b"].astype(np.float64)
    vecs[:, 5] = EPS
    vecs[:, 6:15] = taps
    for h in range(4):
        vecs[:, 15 + h] = fc1b_eff[128 * h:128 * (h + 1)]

    return {
        "wqkv": np.ascontiguousarray(wqkv, np.float32),
        "wproj": np.ascontiguousarray(proj_w, np.float32),
        "wfc1": np.ascontiguousarray(wfc1, np.float32),
        "wfc2": np.ascontiguousarray(fc2_w, np.float32),
        "vecs": np.ascontiguousarray(vecs, np.float32),
    }


_CACHE = {}


class _Bacc(bacc.Bacc):
    """Bacc with the combined Ln+Exp activation-table set preferred, so the
    attention's Exp/Ln/Exp sequence stays on one table (the default
    first-match ordering alternates exp_and_others / natural_log and inserts
    a table load per activation)."""

    def insert_act_table_loads(self):
        import concourse.mybir as _mb
        from concourse.hw_specs import get_activation_tables as _gat
        from concourse.bacc import _bass_rust as _br
        has_activation = any(
            isinstance(i, _mb.InstActivation)
            for b in self.main_func.blocks
            for i in b.instructions
        )
        if not has_activation:
            return
        tables = list(_gat(self.m.arch).items())
        # Keep list order (set ids are positional); strip Exp/Ln from the
        # sets that precede natural_log_exp_and_others so first-match picks
        # the combined set for both.
        out = []
        for name, fns in tables:
            if name == "natural_log_exp_and_others":
                out.append((name, fns))
                continue
            if name in ("exp_and_others", "natural_log"):
                fns = {f for f in fns
                       if getattr(f, "name", str(f)) not in ("Exp", "Ln")}
            out.append((name, fns))
        _br.insert_act_table_loads(self, out)


def _get_nc(dbg=()):
    key = tuple(dbg)
    if key not in _CACHE:
        nc = _Bacc()
        build(nc, dbg)
        nc.finalize()
        _CACHE[key] = nc
    return _CACHE[key]


def kernel(**inputs):
    nc = _get_nc(_DBG[0] if _DBG else ())
    w = _prep_inputs(inputs)
    x = np.asarray(inputs["x"], np.float32)
    in_maps = []
    for c in range(N_CORES):
        m = dict(w)
        m["x"] = np.ascontiguousarray(x[c * IMG:(c + 1) * IMG].reshape(T, C))
        in_maps.append(m)
    trace = os.environ.get("KER_TRACE", "0") == "1"
    kw = {}
    td = os.environ.get("KER_TMPDIR")
    if td:
        kw["tmpdir"] = td
    r = run_bass_kernel_spmd(nc, in_maps, list(range(N_CORES)), trace=trace, **kw)
    out = np.concatenate([r.results[c]["out"].reshape(IMG, L, C)
                          for c in range(N_CORES)], axis=0)
    kernel.last_results = r
    return out


_DBG = []



# revision 60
# speedup vs baseline: 1.1980x; 1.1980x over previous
"""CSWin Transformer block kernel for 8 Trainium2 NeuronCores.

Data-parallel over batch: 32 images -> 4 per core. Each core runs the full
block (LN1, qkv, cross-shaped window attention with LePE, proj, residual,
LN2, MLP, residual) on its shard, fully pipelined per image.

Layouts per core (T = 4*3136 = 12544 tokens):
  - token-major: (112 tokens on partitions, 128 ch free), 28 tiles per image.
  - channel-major: (128 ch on partitions, tokens free).
  - Branch 0 (56x2 column stripes) tokens are kept in w-major order
    (p = 56*w + h) in rows 0:64 of channel-major tensors; branch 1 rows
    64:128 use h-major (t = 56*h + w). Window w of either branch is then
    columns [112*w, 112*w+112).

LN gammas folded into the following matmul weights host-side; LN betas enter
as constant rows via per-partition bias adds on C-major evacuations.
LePE conv bias + the v-bias row are folded into the proj bias.
"""
import sys
sys.path.insert(0, "/opt/trn_rl_repo")
import os
import numpy as np
import concourse.bass as bass
from concourse import bacc
import concourse.tile as tile
from concourse import mybir
from concourse.bass_utils import run_bass_kernel_spmd
from concourse.masks import make_identity

F32 = mybir.dt.float32
BF16 = mybir.dt.bfloat16
AL = mybir.AluOpType
AF = mybir.ActivationFunctionType

N_CORES = 8
B, RESO, C = 32, 56, 128
L = RESO * RESO            # 3136
IMG = B // N_CORES         # 4 images per core
T = IMG * L                # 12544 tokens per core
PT = 112                   # tokens per token-major tile
NTI = L // PT              # 28 token tiles per image
CK = 448                   # tokens per C-major chunk
NCK = L // CK              # 7 chunks per image
NWIN = 28                  # windows per image per branch
WT = 112                   # tokens per window
HD = 32
EPS = 1e-5


def build(nc, dbg=()):
    x_in = nc.declare_dram_parameter("x", [T, C], F32, isOutput=False)
    wqkv_in = nc.declare_dram_parameter("wqkv", [C, 3 * C], F32, isOutput=False)
    wproj_in = nc.declare_dram_parameter("wproj", [C, C], F32, isOutput=False)
    wfc1_in = nc.declare_dram_parameter("wfc1", [C, 4 * C], F32, isOutput=False)
    wfc2_in = nc.declare_dram_parameter("wfc2", [4 * C, C], F32, isOutput=False)
    # vecs cols: 0:s2q 1:s2k 2:s2v 3:projb 4:fc2b 5:eps 6..14:taps 15..18:fc1b
    vecs_in = nc.declare_dram_parameter("vecs", [C, 19], F32, isOutput=False)
    out_t = nc.declare_dram_parameter("out", [T, C], F32, isOutput=True)
    dbg_outs = {}
    for name, shape in dbg:
        dbg_outs[name] = nc.declare_dram_parameter(name, shape, F32, isOutput=True)

    tc = tile.TileContext(nc)
    with tc:
        with (
            tc.tile_pool(name="consts", bufs=1) as consts,
            tc.tile_pool(name="glob", bufs=1) as glob,
            tc.tile_pool(name="pimg", bufs=2) as pimg,
            tc.tile_pool(name="small", bufs=2) as small,
            tc.tile_pool(name="psU", bufs=3, space="PSUM") as psU,
            tc.tile_pool(name="psT", bufs=2, space="PSUM") as psT,
        ):
            _body(nc, consts, glob, pimg, small, psU, psT,
                  x_in, wqkv_in, wproj_in, wfc1_in, wfc2_in, vecs_in,
                  out_t, dbg_outs)
    return nc


def _body(nc, consts, glob, pimg, small, psU, psT,
          x_in, wqkv_in, wproj_in, wfc1_in, wfc2_in, vecs_in, out_t, dbg_outs):
    # ---------------- constants / weights ----------------
    identb = consts.tile([128, 128], BF16)
    make_identity(nc, identb[:])
    onesb = consts.tile([WT, 32], BF16)
    nc.vector.memset(onesb[:], 1.0)
    wqkv = consts.tile([C, 3 * C], BF16)
    nc.gpsimd.dma_start(out=wqkv[:], in_=wqkv_in[:])
    wproj = consts.tile([C, C], BF16)
    nc.gpsimd.dma_start(out=wproj[:], in_=wproj_in[:])
    wfc1 = consts.tile([C, 4 * C], BF16)
    nc.gpsimd.dma_start(out=wfc1[:], in_=wfc1_in[:])
    wfc2 = consts.tile([C, 4, C], BF16)
    nc.gpsimd.dma_start(out=wfc2[:], in_=wfc2_in.rearrange("(k p) o -> p k o", p=C))
    vecs = consts.tile([C, 19], F32)
    nc.sync.dma_start(out=vecs[:], in_=vecs_in[:])
    s2q, s2k, s2v = vecs[:, 0:1], vecs[:, 1:2], vecs[:, 2:3]
    projb, fc2b, epsv = vecs[:, 3:4], vecs[:, 4:5], vecs[:, 5:6]
    taps = [vecs[:, 6 + i:7 + i] for i in range(9)]
    fc1b = [vecs[:, 15 + h:16 + h] for h in range(4)]

    # block-diagonal q: bd_q[p, win, h, :] = q[p, win cols] iff p//32 == h,
    # so ONE matmul per window computes all 4 heads' QK^T (the zeros kill
    # cross-head/cross-branch contraction terms). Diagonal blocks are
    # refilled per image by SBUF->SBUF DMA from the dual-order qT (whose
    # window columns are contiguous); the off-diagonal zeros are written
    # once here and never touched again.
    bd_q = glob.tile([C, NWIN, 4, WT], BF16)
    nc.gpsimd.memset(bd_q[:], 0.0)

    def prefetch_x(img):
        # x in token-major tiles (tile ti = tokens [112*ti, +112) of this image)
        x_tm = pimg.tile([PT, NTI, C], F32, tag="x_img", bufs=3)
        nc.sync.dma_start(
            out=x_tm[:, :, :],
            in_=x_in[img * L:(img + 1) * L].rearrange("(n p) c -> p n c", p=PT))
        return x_tm

    def proj_res1_chunk(ck, stPR):
        # proj + residual-1 for one 448-token chunk of the PREVIOUS image;
        # interwoven with the next image's LN1 groups so the tensor engine
        # has ready matmuls while LN1's vector work gates its transposes.
        x_tm, att_h, sum1, sumsq = stPR
        sl = bass.ts(ck, CK)
        pp = psU.tile([C, 2, 512], F32, tag="u")
        nc.tensor.matmul(pp[:, 0, 0:CK], wproj[:], att_h[:, sl],
                         start=True, stop=True)
        projTc = small.tile([C, CK], BF16, tag="projTc", bufs=3)
        nc.scalar.activation(projTc[:], pp[:, 0, 0:CK], AF.Identity, bias=projb)
        for tj in range(4):
            ti = 4 * ck + tj
            ptp = psT.tile([PT, C], BF16, tag="tp")
            nc.tensor.transpose(ptp[:], projTc[:, bass.ts(tj, PT)], identb[:, 0:C])
            nc.vector.scalar_tensor_tensor(out=x_tm[:, ti, :], in0=ptp[:],
                                           scalar=1.0, in1=x_tm[:, ti, :],
                                           op0=AL.mult, op1=AL.add,
                                           accum_out=sum1[:, ti:ti + 1])
            sq = small.tile([PT, C], F32, tag="sqt")
            nc.scalar.activation(sq[:], x_tm[:, ti, :], AF.Square,
                                 accum_out=sumsq[:, ti:ti + 1])

    def phase_A1(img, x_tm, stPR=None):
        base_t = 0

        # ---- LN1 stats + apply + transpose ----
        mvs = small.tile([PT, NTI, 2], F32, tag="mvs")
        rstd = small.tile([PT, NTI], F32, tag="rstd")
        lnx_h = pimg.tile([C, L], BF16, tag="lnx_h")
        lnx_w = pimg.tile([C, L], BF16, tag="lnx_w")
        lnx_h_v = lnx_h.rearrange("p (n t) -> p n t", t=PT)
        for tg in range(NTI // 7):
            for ti in range(7 * tg, 7 * tg + 7):
                st = small.tile([PT, 6], F32, tag="bnst")
                nc.vector.bn_stats(out=st[:], in_=x_tm[:, base_t + ti, :])
                nc.vector.bn_aggr(out=mvs[:, ti, :], in_=st[:])
            gsl = bass.ds(7 * tg, 7)
            nc.scalar.activation(rstd[:, gsl], mvs[:, gsl, 1], AF.Ln,
                                 bias=epsv[0:PT, :])
            nc.scalar.activation(rstd[:, gsl], rstd[:, gsl], AF.Exp, scale=-0.5)
        # w-major reorder views: p = 56*w + h  <-  t = 56*h + w. The copy is
        # chunked along h (the freshly-evacuated columns), so each reorder
        # piece runs right after its 4-tile group and the whole reorder
        # finishes with LN1 instead of serializing after it.
        lnw_v = lnx_w.rearrange("p (w h) -> p w h", w=RESO)
        lnh_v = lnx_h.rearrange("p (h w) -> p h w", h=RESO).rearrange("p h w -> p w h")
        for tq in range(NTI // 4):
            ztp = psT.tile([C, 4, PT], BF16, tag="tp")
            for j in range(4):
                ti = 4 * tq + j
                z = small.tile([PT, C], BF16, tag="zt")
                nc.vector.tensor_scalar(out=z[:], in0=x_tm[:, base_t + ti, :],
                                        scalar1=mvs[:, ti, 0:1],
                                        scalar2=rstd[:, ti:ti + 1],
                                        op0=AL.subtract, op1=AL.mult)
                nc.tensor.transpose(ztp[:, j, :], z[:], identb[0:PT, 0:PT])
            nc.vector.tensor_copy(lnx_h_v[:, bass.ds(4 * tq, 4), :], ztp[:, :, :])
            hv = bass.ds(8 * tq, 8)
            nc.gpsimd.tensor_copy(out=lnw_v[:, :, hv], in_=lnh_v[:, :, hv])
            if stPR is not None:
                proj_res1_chunk(tq, stPR)

        # ---- qkv (col-packed: br0 from lnx_w -> rows 0:64, br1 from lnx_h).
        # q only feeds the bd_q diagonal refill, so it lives in a small
        # per-chunk tile and the DMAs (idle DMA engines, ~25ns Pool-sequencer
        # dispatch) are issued inline ----
        kT = pimg.tile([C, L], BF16, tag="kT")
        vT = pimg.tile([C, L], BF16, tag="vT")
        for ck in range(NCK):
            sl = bass.ts(ck, CK)
            pqk = psU.tile([C, 2, 512], F32, tag="u")
            pv = psU.tile([C, 2, 512], F32, tag="u")
            for half, src in ((0, lnx_w), (1, lnx_h)):
                hs = bass.ds(64 * half, 64)
                nc.tensor.matmul(pqk[hs, 0, 0:CK], wqkv[:, bass.ds(64 * half, 64)],
                                 src[:, sl], start=True, stop=True,
                                 tile_position=(0, 64 * half))
                nc.tensor.matmul(pqk[hs, 1, 0:CK], wqkv[:, bass.ds(C + 64 * half, 64)],
                                 src[:, sl], start=True, stop=True,
                                 tile_position=(0, 64 * half))
                nc.tensor.matmul(pv[hs, 0, 0:CK], wqkv[:, bass.ds(2 * C + 64 * half, 64)],
                                 src[:, sl], start=True, stop=True,
                                 tile_position=(0, 64 * half))
            qTc = small.tile([C, CK], BF16, tag="qTc", bufs=3)
            nc.scalar.activation(qTc[:], pqk[:, 0, 0:CK], AF.Identity, bias=s2q)
            nc.scalar.activation(kT[:, sl], pqk[:, 1, 0:CK], AF.Identity, bias=s2k)
            nc.vector.tensor_scalar(out=vT[:, sl], in0=pv[:, 0, 0:CK], scalar1=s2v,
                                    scalar2=None, op0=AL.add)
            for h in range(4):
                hp = bass.ds(32 * h, 32)
                nc.gpsimd.dma_start(
                    out=bd_q[hp, bass.ds(4 * ck, 4), h, :],
                    in_=qTc[hp, :].rearrange("p (g q) -> p g q", g=4))

        # ---- v_tm: token-major v via transposes of dual-order vT (window
        # columns are contiguous), 4 windows per psum bank ----
        v_tm = pimg.tile([PT, NWIN, C], BF16, tag="v_tm")
        for r in range(NWIN // 4):
            pvt = psT.tile([PT, 4, C], BF16, tag="tp")
            for g in range(4):
                nc.tensor.transpose(pvt[:, g, :], vT[:, bass.ts(4 * r + g, WT)],
                                    identb[:])
            nc.vector.tensor_copy(v_tm[:, bass.ds(4 * r, 4), :], pvt[:, :, :])
        return x_tm, kT, vT, v_tm

    def phase_A2(img, stateA1):
        x_tm, kT, vT, v_tm = stateA1
        # ---- attention: per superchunk (4 windows); one QK matmul per
        # window computes all 4 heads via the block-diagonal q. LePE taps
        # and the h-major reorder are chunked per superchunk too (windows
        # are independent for the depthwise conv), so they pipeline under
        # later superchunks' matmuls instead of serializing at the end ----
        tmp_att = pimg.tile([C, L], BF16, tag="tmp_att")
        att_h = pimg.tile([C, L], BF16, tag="att_h")
        va = vT.rearrange("p (s y x) -> p s y x", s=NWIN, y=2)
        aa = tmp_att.rearrange("p (s y x) -> p s y x", s=NWIN, y=2)
        ah_v = att_h[0:64].rearrange("p (h w) -> p h w", h=RESO)
        ta_v = tmp_att[0:64].rearrange("p (w h) -> p w h", w=RESO).rearrange(
            "p w h -> p h w")

        def tap(dy, dx, ssl):
            t = taps[(dy + 1) * 3 + (dx + 1)]
            xo0, xo1 = max(0, -dx), max(0, dx)
            xn = RESO - abs(dx)
            if dy == 0:
                o = aa[:, ssl, :, bass.ds(xo0, xn)]
                i = va[:, ssl, :, bass.ds(xo1, xn)]
            elif dy == 1:
                o = aa[:, ssl, 0:1, bass.ds(xo0, xn)]
                i = va[:, ssl, 1:2, bass.ds(xo1, xn)]
            else:
                o = aa[:, ssl, 1:2, bass.ds(xo0, xn)]
                i = va[:, ssl, 0:1, bass.ds(xo1, xn)]
            nc.vector.scalar_tensor_tensor(out=o, in0=i, scalar=t, in1=o,
                                           op0=AL.mult, op1=AL.add)

        for s4 in range(NCK):
            # pT_t[tk, h, win-in-s4, tq]
            pT_t = small.tile([WT, 4, 4, WT], BF16, tag="pT")
            for wp in range(2):            # window pairs
                sp = psU.tile([WT, 2, 512], F32, tag="u")
                for g2 in range(2):
                    win = 4 * s4 + 2 * wp + g2
                    nc.tensor.matmul(sp[:, g2, 0:4 * WT], kT[:, bass.ts(win, WT)],
                                     bd_q[:, win, :, :], start=True, stop=True)
                # psum cols are (h, tq) per window; view to pT_t (h, win, tq)
                spv = sp[:, :, 0:4 * WT].rearrange("p w (h q) -> p h w q", h=4)
                nc.scalar.activation(pT_t[:, :, bass.ds(2 * wp, 2), :],
                                     spv, AF.Exp)
            sa = psU.tile([C, 2, 512], F32, tag="u")
            sums = sa[:, 0, 0:CK]
            avp = sa[:, 1, 0:CK]
            for h in range(4):
                po = bass.ds(32 * h, 32)
                nc.tensor.matmul(sa[po, 0, 0:CK], onesb[:, 0:32], pT_t[:, h, :, :],
                                 start=True, stop=True, tile_position=(0, 32 * h))
                vsl = bass.ds(64 * (h // 2) + 32 * (h % 2), 32)
                for g in range(4):
                    nc.tensor.matmul(sa[po, 1, bass.ds(112 * g, WT)],
                                     v_tm[:, 4 * s4 + g, vsl],
                                     pT_t[:, h, g, :],
                                     start=True, stop=True,
                                     tile_position=(0, 32 * h))
            lg = small.tile([C, CK], F32, tag="lg")
            nc.scalar.activation(lg[:], sums, AF.Ln)
            rec = small.tile([C, CK], F32, tag="rec")
            nc.scalar.activation(rec[:], lg[:], AF.Exp, scale=-1.0)
            nc.vector.tensor_tensor(out=tmp_att[:, bass.ts(s4, CK)], in0=avp,
                                    in1=rec[:], op=AL.mult)
            # LePE taps for these 4 stripes, then reorder them to h-major
            ssl = bass.ds(4 * s4, 4)
            for dy in (0, 1, -1):
                for dx in (0, 1, -1):
                    tap(dy, dx, ssl)
            wv = bass.ds(8 * s4, 8)
            nc.gpsimd.tensor_copy(out=ah_v[:, :, wv], in_=ta_v[:, :, wv])
            nc.gpsimd.tensor_copy(out=att_h[64:128, bass.ts(s4, CK)],
                                  in_=tmp_att[64:128, bass.ts(s4, CK)])

        if "attT" in dbg_outs and img == 0:
            dc = small.tile([C, L], F32, tag="dbg")
            nc.vector.tensor_copy(dc[:], att_h[:])
            nc.sync.dma_start(out=dbg_outs["attT"], in_=dc[:])
        return x_tm, att_h

    def phase_B1(img, state, stPR):
        x_tm, att_h = state
        _, _, sum1, sumsq = stPR
        base_t = 0
        # rstd2 = 1/sqrt(sumsq/C - (sum1/C)^2 + eps); mean2 = sum1/C
        mean2 = small.tile([PT, NTI], F32, tag="mean2")
        var2 = small.tile([PT, NTI], F32, tag="var2")
        rstd2 = small.tile([PT, NTI], F32, tag="rstd2")
        lnx2 = pimg.tile([C, L], BF16, tag="lnx2")
        for tg in range(NTI // 7):
            gsl = bass.ds(7 * tg, 7)
            nc.vector.tensor_scalar(out=mean2[:, gsl], in0=sum1[:, gsl],
                                    scalar1=1.0 / C, scalar2=None, op0=AL.mult)
            nc.vector.tensor_tensor(out=var2[:, gsl], in0=mean2[:, gsl],
                                    in1=mean2[:, gsl], op=AL.mult)
            nc.vector.scalar_tensor_tensor(out=var2[:, gsl], in0=sumsq[:, gsl],
                                           scalar=1.0 / C, in1=var2[:, gsl],
                                           op0=AL.mult, op1=AL.subtract)
            nc.scalar.activation(rstd2[:, gsl], var2[:, gsl], AF.Ln,
                                 bias=epsv[0:PT, :])
            nc.scalar.activation(rstd2[:, gsl], rstd2[:, gsl], AF.Exp, scale=-0.5)
        lnx2_v = lnx2.rearrange("p (n t) -> p n t", t=PT)
        for tq in range(NTI // 4):
            z2p = psT.tile([C, 4, PT], BF16, tag="tp")
            for j in range(4):
                ti = 4 * tq + j
                z2 = small.tile([PT, C], BF16, tag="z2t")
                nc.vector.tensor_scalar(out=z2[:], in0=x_tm[:, base_t + ti, :],
                                        scalar1=mean2[:, ti:ti + 1],
                                        scalar2=rstd2[:, ti:ti + 1],
                                        op0=AL.subtract, op1=AL.mult)
                nc.tensor.transpose(z2p[:, j, :], z2[:], identb[0:PT, 0:PT])
            nc.vector.tensor_copy(lnx2_v[:, bass.ds(4 * tq, 4), :], z2p[:, :, :])
        return lnx2

    def phase_B2(img, x_tm, lnx2):
        base_t = 0
        # ---- MLP + residual 2 ----
        for ck in range(NCK):
            sl = bass.ts(ck, CK)
            hb = small.tile([C, 4, CK], BF16, tag="hb")
            for hp in range(2):
                ph = psU.tile([C, 2, 512], F32, tag="u")
                for hh in range(2):
                    h = 2 * hp + hh
                    nc.tensor.matmul(ph[:, hh, 0:CK], wfc1[:, bass.ds(128 * h, 128)],
                                     lnx2[:, sl], start=True, stop=True)
                    nc.scalar.activation(hb[:, h, :], ph[:, hh, 0:CK], AF.Gelu,
                                         bias=fc1b[h])
            p2 = psU.tile([C, 2, 512], F32, tag="u")
            for h in range(4):
                nc.tensor.matmul(p2[:, 0, 0:CK], wfc2[:, h, :], hb[:, h, :],
                                 start=(h == 0), stop=(h == 3))
            f2 = small.tile([C, CK], BF16, tag="f2")
            nc.scalar.activation(f2[:], p2[:, 0, 0:CK], AF.Identity, bias=fc2b)
            ftp = psT.tile([PT, 4, C], BF16, tag="tp")
            for tj in range(4):
                nc.tensor.transpose(ftp[:, tj, :], f2[:, bass.ts(tj, PT)],
                                    identb[:, 0:C])
            xsl = x_tm[:, bass.ds(base_t + 4 * ck, 4), :]
            nc.vector.tensor_tensor(out=xsl, in0=ftp[:, :, :], in1=xsl, op=AL.add)

        # out on the Activation HWDGE queue: keeps the sync queue free for
        # the x prefetches so neither stream head-of-line blocks the other
        nc.scalar.dma_start(
            out=out_t[img * L:(img + 1) * L].rearrange("(n p) c -> p n c", p=PT),
            in_=x_tm[:, :, :])

    # Skewed software pipeline, zippered at half-phase granularity: each
    # step emits A1(i), B1(i-1), A2(i), B2(i-1) so every engine's in-order
    # queue alternates between independent work from adjacent images —
    # halving head-of-line blocking versus whole-phase interleaving. The
    # scalar stream still ends each step with the gelu block, so the
    # exp/ln <-> gelu activation-table swap stays at 2 loads per image.
    stA1 = [None] * IMG
    stA2 = [None] * IMG
    stB1 = [None] * IMG
    stPR = [None] * IMG
    xs = [None] * IMG
    xs[0] = prefetch_x(0)
    for i in range(IMG + 1):
        if i >= 1:
            # proj/res1 state for image i-1 (filled chunk-wise inside A1(i))
            sum1 = small.tile([PT, NTI], F32, tag="sum1", name="sum1")
            sumsq = small.tile([PT, NTI], F32, tag="sumsq", name="sumsq")
            stPR[i - 1] = (stA2[i - 1][0], stA2[i - 1][1], sum1, sumsq)
        if i < IMG:
            stA1[i] = phase_A1(i, xs[i], stPR[i - 1] if i >= 1 else None)
            # prefetch the next image's x now; with 3 x buffers the slot
            # being refilled belongs to image i-2, whose B2 finished a full
            # step ago, so the DMA starts immediately
            if i + 1 < IMG:
                xs[i + 1] = prefetch_x(i + 1)
        else:
            # last image has no following A1 to host its proj/res1 chunks
            for ck in range(NCK):
                proj_res1_chunk(ck, stPR[i - 1])
        if i >= 1:
            stB1[i - 1] = phase_B1(i - 1, stA2[i - 1], stPR[i - 1])
        if i < IMG:
            stA2[i] = phase_A2(i, stA1[i])
        if i >= 1:
            phase_B2(i - 1, stA2[i - 1][0], stB1[i - 1])


def _prep_inputs(inputs):
    """Host-side weight preprocessing (fp64 for exact folds)."""
    g1 = inputs["norm1_g"].astype(np.float64)
    b1 = inputs["norm1_b"].astype(np.float64)
    g2 = inputs["norm2_g"].astype(np.float64)
    b2 = inputs["norm2_b"].astype(np.float64)
    qkv_w = inputs["qkv_w"].astype(np.float64)
    proj_w = inputs["proj_w"].astype(np.float64)
    fc1_w = inputs["fc1_w"].astype(np.float64)
    fc2_w = inputs["fc2_w"].astype(np.float64)
    scale = HD ** -0.5

    wqkv = g1[:, None] * qkv_w
    s2 = b1 @ qkv_w
    wqkv[:, 0:C] *= scale
    s2q = s2[0:C] * scale
    s2k = s2[C:2 * C]
    s2v = s2[2 * C:3 * C]

    # LePE taps in stripe coords (y = stripe row in {0,1}, x = along stripe):
    # br1 (rows 64:128, h-major): (y,x) = (img_y, img_x) -> w1[dy+1, dx+1]
    # br0 (rows 0:64, w-major):  (y,x) = (img_x, img_y) -> transposed kernel
    w0 = inputs["conv_w0"].astype(np.float64)[:, 0]
    w1 = inputs["conv_w1"].astype(np.float64)[:, 0]
    taps = np.zeros((C, 9))
    for dy in (-1, 0, 1):
        for dx in (-1, 0, 1):
            ti = (dy + 1) * 3 + (dx + 1)
            taps[0:64, ti] = w0[:, dx + 1, dy + 1]
            taps[64:128, ti] = w1[:, dy + 1, dx + 1]

    # v_tm is transposed from vT, which already carries s2v, so unlike the
    # conv bias it must not be folded into the proj bias here.
    cb = np.concatenate([inputs["conv_b0"], inputs["conv_b1"]]).astype(np.float64)
    projb_eff = inputs["proj_b"].astype(np.float64) + cb @ proj_w

    wfc1 = g2[:, None] * fc1_w
    fc1b_eff = b2 @ fc1_w + inputs["fc1_b"].astype(np.float64)

    vecs = np.zeros((C, 19))
    vecs[:, 0], vecs[:, 1], vecs[:, 2] = s2q, s2k, s2v
    vecs[:, 3], vecs[:, 4] = projb_eff, inputs["fc2_b"].astype(np.float64)
    vecs[:, 5] = EPS
    vecs[:, 6:15] = taps
    for h in range(4):
        vecs[:, 15 + h] = fc1b_eff[128 * h:128 * (h + 1)]

    return {
        "wqkv": np.ascontiguousarray(wqkv, np.float32),
        "wproj": np.ascontiguousarray(proj_w, np.float32),
        "wfc1": np.ascontiguousarray(wfc1, np.float32),
        "wfc2": np.ascontiguousarray(fc2_w, np.float32),
        "vecs": np.ascontiguousarray(vecs, np.float32),
    }


_CACHE = {}


class _Bacc(bacc.Bacc):
    """Bacc with the combined Ln+Exp activation-table set preferred, so the
    attention's Exp/Ln/Exp sequence stays on one table (the default
    first-match ordering alternates exp_and_others / natural_log and inserts
    a table load per activation)."""

    def insert_act_table_loads(self):
        import concourse.mybir as _mb
        from concourse.hw_specs import get_activation_tables as _gat
        from concourse.bacc import _bass_rust as _br
        has_activation = any(
            isinstance(i, _mb.InstActivation)
            for b in self.main_func.blocks
            for i in b.instructions
        )
        if not has_activation:
            return
        tables = list(_gat(self.m.arch).items())
        # Keep list order (set ids are positional); strip Exp/Ln from the
        # sets that precede natural_log_exp_and_others so first-match picks
        # the combined set for both.
        out = []
        for name, fns in tables:
            if name == "natural_log_exp_and_others":
                out.append((name, fns))
                continue
            if name in ("exp_and_others", "natural_log"):
                fns = {f for f in fns
                       if getattr(f, "name", str(f)) not in ("Exp", "Ln")}
            out.append((name, fns))
        _br.insert_act_table_loads(self, out)


def _get_nc(dbg=()):
    key = tuple(dbg)
    if key not in _CACHE:
        nc = _Bacc()
        build(nc, dbg)
        nc.finalize()
        _CACHE[key] = nc
    return _CACHE[key]


def kernel(**inputs):
    nc = _get_nc(_DBG[0] if _DBG else ())
    w = _prep_inputs(inputs)
    x = np.asarray(inputs["x"], np.float32)
    in_maps = []
    for c in range(N_CORES):
        m = dict(w)
        m["x"] = np.ascontiguousarray(x[c * IMG:(c + 1) * IMG].reshape(T, C))
        in_maps.append(m)
    trace = os.environ.get("KER_TRACE", "0") == "1"
    kw = {}
    td = os.environ.get("KER_TMPDIR")
    if td:
        kw["tmpdir"] = td
    r = run_bass_kernel_spmd(nc, in_maps, list(range(N_CORES)), trace=trace, **kw)
    out = np.concatenate([r.results[c]["out"].reshape(IMG, L, C)
                          for c in range(N_CORES)], axis=0)
    kernel.last_results = r
    return out


_DBG = []

